# revision 10
# baseline (speedup 1.0000x reference)
"""ATR (twin-gate RNN) Trainium2 kernel.

  p = x @ W1.T + b1                       (batched GEMM over all T)
  h_t = sig(p_t+q_t)*p_t + sig(p_t-q_t)*q_t,  q_t = h_{t-1} @ W2.T + b2

Sharding: data-parallel over batch B=128 -> 16 per core across 8 NeuronCores,
zero cross-core communication. All tensors are kept on-device in a
[d-partition, batch-free] (transposed) layout; host prepares/unprepares.

Design notes (v2):
- b2 is folded into a PSUM pre-fill (constant copied into the accumulator
  ring every 4 steps); recurrence matmuls accumulate on top (start=False),
  so PSUM holds qhat = W2 h + b2 directly.
- p stored t-major [P, T, DT, BL] so all per-step reads are contiguous.
- qhat copied PSUM->SBUF once per half (ACT for A, DVE for B); the gate
  chain (s, d, t1, t2, h) then runs in fast 2x SBUF fp16 mode on DVE.
- Output h staged in SBUF, DMA'd every OG=4 steps (big transfers).
- Recurrence MMs ordered dl-outer, A-half-first: next step's first MMs
  depend only on hA.
- Junk matmuls pad the PE queue so the HAM clock gate stays at 2.4 GHz.
"""

import contextlib

import numpy as np

import concourse.bass as bass
import concourse.mybir as mybir
import concourse.tile as tile
from concourse import bacc
from concourse.bass import ts
from concourse.bass_utils import run_bass_kernel_spmd
from concourse.tile import add_dep_helper


def _chain(insts, reason):
    """Force issue order within an engine queue (no extra semaphores).

    add_dep_helper(x, y) makes x wait on y, so the later op is the first arg.
    """
    insts = [i for i in insts if i is not None]
    for a, b in zip(insts, insts[1:]):
        add_dep_helper(b.ins, a.ins, sync=False, reason=reason)
    return insts


B, T, D = 128, 256, 1024
NCORES = 8
BL = B // NCORES          # 16 batches per core
P = 128                   # partitions
DT = D // P               # 8 d-tiles
HDT = DT // 2             # 4 (half)
TW = 32                   # t-window per p-GEMM chunk (TW*BL = 512 cols)
NCH = T // TW             # 8 chunks
N_UP = 2                  # upfront t-windows for stage 1
OG = 4                    # out-DMA grouping (steps per DMA)
F16 = mybir.dt.float16
F32 = mybir.dt.float32

_CACHE = {}


def _emit(nc, xT, w1, w2, b1, b2bc4, h0, outT):
    tc = nc._tc
    SIG = mybir.ActivationFunctionType.Sigmoid
    IDENT = mybir.ActivationFunctionType.Identity
    with contextlib.ExitStack() as ctx:
        singles = ctx.enter_context(tc.tile_pool(name="singles", bufs=1))
        w1_sb = singles.tile([P, DT, D], F16)
        nc.sync.dma_start(out=w1_sb, in_=w1)
        w2_sb = singles.tile([P, DT, D], F16)
        nc.sync.dma_start(out=w2_sb, in_=w2)
        b1_sb = singles.tile([P, DT], F32)
        nc.sync.dma_start(out=b1_sb, in_=b1)
        b2bc_sb = singles.tile([P, OG, DT, BL], F32)
        nc.sync.dma_start(out=b2bc_sb, in_=b2bc4)
        h0_sb = singles.tile([P, DT, BL], F16)
        nc.sync.dma_start(out=h0_sb, in_=h0)
        # p stored t-major: per-step slice [P, DT, BL] is contiguous
        p_sb = singles.tile([P, T, DT, BL], F16)

        psingles = ctx.enter_context(
            tc.tile_pool(name="psingles", bufs=1, space="PSUM")
        )
        junk_ps = psingles.tile([P, 512], F32)

        xT_r = xT.rearrange("(a p) t b -> a p t b", p=P)

        with contextlib.ExitStack() as rctx:
            xpool = rctx.enter_context(tc.tile_pool(name="xin", bufs=2))
            ppsum = rctx.enter_context(
                tc.tile_pool(name="pps", bufs=2, space="PSUM")
            )
            qpool = rctx.enter_context(
                tc.tile_pool(name="qps", bufs=3, space="PSUM")
            )
            gp = rctx.enter_context(tc.tile_pool(name="gp", bufs=4))
            hp = rctx.enter_context(tc.tile_pool(name="hp", bufs=2))

            xn_tiles = {}
            mmchain = [None]
            dvechain = [None]
            actchain = [None]

            def mm_order(i):
                if mmchain[0] is not None:
                    add_dep_helper(i.ins, mmchain[0].ins, sync=False,
                                   reason="mm-order")
                mmchain[0] = i

            def dve_order(i):
                if dvechain[0] is not None:
                    add_dep_helper(i.ins, dvechain[0].ins, sync=False,
                                   reason="dve-order")
                dvechain[0] = i

            def act_order(i):
                if actchain[0] is not None:
                    add_dep_helper(i.ins, actchain[0].ins, sync=False,
                                   reason="act-order")
                actchain[0] = i

            def load_xn(n, dt_range):
                if n not in xn_tiles:
                    xn_tiles[n] = xpool.tile(
                        [P, DT, TW, BL], F16, tag="xn", name=f"xn_{n}"
                    )
                for dt in dt_range:
                    nc.sync.dma_start(
                        out=xn_tiles[n][:, dt], in_=xT_r[dt, :, ts(n, TW), :]
                    )

            pps_tiles = {}

            def p_mms(n, e, dts):
                if (n, e) not in pps_tiles:
                    pps_tiles[(n, e)] = ppsum.tile(
                        [P, TW * BL], F32, tag="pp", name=f"pp_{n}_{e}"
                    )
                ps = pps_tiles[(n, e)]
                for dt in dts:
                    i_mm = nc.tensor.matmul(
                        ps,
                        lhsT=w1_sb[:, dt, ts(e, P)],
                        rhs=xn_tiles[n][:, dt].rearrange("p t b -> p (t b)"),
                        start=(dt == 0),
                        stop=(dt == DT - 1),
                    )
                    mm_order(i_mm)

            def p_copy(n, e):
                ps = pps_tiles[(n, e)]
                i_a = nc.scalar.activation(
                    out=p_sb[:, ts(n, TW), e, :],
                    in_=ps.rearrange("p (t b) -> p t b", t=TW),
                    func=IDENT,
                    bias=b1_sb[:, e : e + 1],
                    scale=1.0,
                )
                act_order(i_a)
                return i_a

            qq_tiles = {}

            def prefill(g):
                # fresh accumulator tile for steps [OG*g, OG*g+OG),
                # pre-filled with b2 so the MMs accumulate qhat = W2 h + b2
                qq_tiles[g] = qpool.tile(
                    [P, OG, DT, BL], F32, tag="qq", name=f"qq_{g}"
                )
                i = nc.vector.tensor_copy(out=qq_tiles[g], in_=b2bc_sb)
                dve_order(i)
                return i

            def junk_mm(t, j):
                i_mm = nc.tensor.matmul(
                    junk_ps,
                    lhsT=w1_sb[:, 0, 0:P],
                    rhs=p_sb[:, 0:OG].rearrange("p t d b -> p (t d b)"),
                    start=True,
                    stop=True,
                    skip_group_check=True,
                )
                mm_order(i_mm)

            # ---- stage-1 upfront: chunks 0..N_UP-1
            for n in range(N_UP):
                load_xn(n, range(DT))
                for e in range(DT):
                    p_mms(n, e, range(DT))
                    p_copy(n, e)
            load_xn(N_UP, range(DT))
            prefill(0)

            # per-step schedule for interleaved stage-1 chunks
            step_mms = {}     # t -> (n, e, [dt, dt])
            step_copies = {}  # t -> (n, e)
            step_dma = {}     # t -> (n, [dts])
            K0 = 8
            for k in range((NCH - N_UP) * DT):
                n, e = N_UP + k // DT, k % DT
                for j in range(4):
                    step_mms[K0 + 4 * k + j] = (n, e, [2 * j, 2 * j + 1])
                step_copies[K0 + 4 * k + 3] = (n, e)
            for n in range(N_UP + 1, NCH):
                base = 32 * n - 72
                for dt in range(DT):
                    step_dma[base + dt] = (n, [dt])

            hprev = h0_sb  # [P, DT, BL] view for step-t rhs
            hstages = {}

            for t in range(T):
                g = t // OG
                slot = t % OG
                if slot == 0:
                    hstages[g] = hp.tile(
                        [P, OG, DT, BL], F16, tag="hst", name=f"hst_{g}"
                    )
                hst = hstages[g]
                qq = qq_tiles[g][:, slot]  # [P, DT, BL] f32 psum

                # ---- recurrence MMs: A-block (e0..3) then B-block (e4..7),
                # dl-outer so the first 16 MMs of each block depend only on
                # the previous step's A-half of h.
                for eoff in (0, HDT):
                    for dl in range(DT):
                        for el in range(HDT):
                            i_mm = nc.tensor.matmul(
                                qq[:, eoff + el, :],
                                lhsT=w2_sb[:, dl, ts(eoff + el, P)],
                                rhs=hprev[:, dl, :],
                                start=False,
                                stop=(dl == DT - 1),
                                skip_group_check=True,
                            )
                            mm_order(i_mm)

                # ---- gate chain
                qqA = qq[:, 0:HDT, :]
                qqB = qq[:, HDT:DT, :]
                pA = p_sb[:, t, 0:HDT, :]
                pB = p_sb[:, t, HDT:DT, :]

                qcA = gp.tile([P, HDT, BL], F16, tag="qcA")
                i_qcA = nc.scalar.copy(out=qcA, in_=qqA)
                qcB = gp.tile([P, HDT, BL], F16, tag="qcB")
                i_qcB = nc.vector.tensor_copy(out=qcB, in_=qqB)

                sdA = gp.tile([P, 2, HDT, BL], F16, tag="sdA")
                i_sA = nc.vector.tensor_add(sdA[:, 0], pA, qcA)
                i_dA = nc.vector.tensor_sub(sdA[:, 1], pA, qcA)
                igfgA = gp.tile([P, 2, HDT, BL], F16, tag="igfgA")
                i_sigA = nc.scalar.activation(
                    out=igfgA.rearrange("p s d b -> p (s d b)"),
                    in_=sdA.rearrange("p s d b -> p (s d b)"),
                    func=SIG,
                )
                sdB = gp.tile([P, 2, HDT, BL], F16, tag="sdB")
                i_sB = nc.vector.tensor_add(sdB[:, 0], pB, qcB)
                i_dB = nc.vector.tensor_sub(sdB[:, 1], pB, qcB)
                igfgB = gp.tile([P, 2, HDT, BL], F16, tag="igfgB")
                i_sigB = nc.scalar.activation(
                    out=igfgB.rearrange("p s d b -> p (s d b)"),
                    in_=sdB.rearrange("p s d b -> p (s d b)"),
                    func=SIG,
                )

                t1A = gp.tile([P, HDT, BL], F16, tag="t1A")
                i_t1A = nc.vector.tensor_mul(t1A, igfgA[:, 0], pA)
                t2A = gp.tile([P, HDT, BL], F16, tag="t2A")
                i_t2A = nc.vector.tensor_mul(t2A, igfgA[:, 1], qcA)
                i_hA = nc.vector.tensor_add(hst[:, slot, 0:HDT, :], t1A, t2A)
                t1B = gp.tile([P, HDT, BL], F16, tag="t1B")
                i_t1B = nc.vector.tensor_mul(t1B, igfgB[:, 0], pB)
                t2B = gp.tile([P, HDT, BL], F16, tag="t2B")
                i_t2B = nc.vector.tensor_mul(t2B, igfgB[:, 1], qcB)
                i_hB = nc.vector.tensor_add(hst[:, slot, HDT:DT, :], t1B, t2B)

                # allocate + prefill the next group's accumulator one group
                # ahead (pool rotation handles the WAR hazards cleanly)
                if slot == 0 and g + 1 <= (T - 1) // OG:
                    prefill(g + 1)

                # interleaved stage-1 work
                if t in step_dma:
                    nd, dts = step_dma[t]
                    load_xn(nd, dts)
                n_junk = 2
                if t in step_mms:
                    np_, ep_, dts = step_mms[t]
                    p_mms(np_, ep_, dts)
                else:
                    n_junk = 4
                cp = p_copy(*step_copies[t]) if t in step_copies else None
                for j in range(n_junk):
                    junk_mm(t, j)

                # engine issue order
                for i in (i_sA, i_dA, i_qcB, i_sB, i_dB,
                          i_t1A, i_t2A, i_hA, i_t1B, i_t2B, i_hB):
                    dve_order(i)
                for i in (i_qcA, i_sigA, i_sigB):
                    act_order(i)

                # out DMA every OG steps
                if slot == OG - 1:
                    nc.sync.dma_start(out=outT[g], in_=hst)

                hprev = hst[:, slot]


def build():
    if "nc" in _CACHE:
        return _CACHE["nc"]
    nc = bacc.Bacc("TRN2", target_bir_lowering=False, debug=False,
                   num_devices=NCORES)
    xT = nc.dram_tensor("xT", [D, T, BL], F16, kind="ExternalInput").ap()
    w1 = nc.dram_tensor("w1", [P, DT, D], F16, kind="ExternalInput").ap()
    w2 = nc.dram_tensor("w2", [P, DT, D], F16, kind="ExternalInput").ap()
    b1 = nc.dram_tensor("b1", [P, DT], F32, kind="ExternalInput").ap()
    b2bc4 = nc.dram_tensor("b2bc4", [P, OG, DT, BL], F32,
                           kind="ExternalInput").ap()
    h0 = nc.dram_tensor("h0", [P, DT, BL], F16, kind="ExternalInput").ap()
    outT = nc.dram_tensor("outT", [T // OG, P, OG, DT, BL], F16,
                          kind="ExternalOutput").ap()
    with tile.TileContext(nc) as tc:
        nc._tc = tc
        _emit(nc, xT, w1, w2, b1, b2bc4, h0, outT)
    nc.compile()
    _CACHE["nc"] = nc
    return nc


def make_in_maps(x, W1, b1, W2, b2, init_hx):
    x = np.asarray(x, dtype=np.float32)
    W1 = np.asarray(W1, dtype=np.float32)
    b1 = np.asarray(b1, dtype=np.float32)
    W2 = np.asarray(W2, dtype=np.float32)
    b2 = np.asarray(b2, dtype=np.float32)
    init_hx = np.asarray(init_hx, dtype=np.float32)

    w1s = np.ascontiguousarray(
        W1.T.reshape(DT, P, D).transpose(1, 0, 2)
    ).astype(np.float16)  # [din, dtile, e] = W1[e, d]
    w2s = np.ascontiguousarray(
        W2.T.reshape(DT, P, D).transpose(1, 0, 2)
    ).astype(np.float16)
    b1s = np.ascontiguousarray(b1.reshape(DT, P).T)  # [e_in, e_tile]
    b2v = np.ascontiguousarray(b2.reshape(DT, P).T)  # [P, DT]
    b2bc4 = np.ascontiguousarray(
        np.broadcast_to(b2v[:, None, :, None], (P, OG, DT, BL))
    ).astype(np.float32)
    h0 = np.ascontiguousarray(
        np.broadcast_to(init_hx.reshape(DT, P).T[:, :, None], (P, DT, BL))
    ).astype(np.float16)

    in_maps = []
    for c in range(NCORES):
        xc = x[c * BL : (c + 1) * BL]  # [BL, T, D]
        xTc = np.ascontiguousarray(xc.transpose(2, 1, 0)).astype(np.float16)
        in_maps.append(
            {"xT": xTc, "w1": w1s, "w2": w2s, "b1": b1s, "b2bc4": b2bc4,
             "h0": h0}
        )
    return in_maps


def assemble(results):
    out = np.empty((B, T, D), dtype=np.float32)
    for c in range(NCORES):
        oT = results[c]["outT"]  # [T//OG, P, OG, DT, BL] f16
        out[c * BL : (c + 1) * BL] = (
            oT.transpose(4, 0, 2, 3, 1).reshape(BL, T, D).astype(np.float32)
        )
    return out


def run(inputs, trace=False):
    nc = build()
    in_maps = make_in_maps(**inputs)
    res = run_bass_kernel_spmd(nc, in_maps, list(range(NCORES)), trace=trace)
    return assemble(res.results), res


def kernel(x, W1, b1, W2, b2, init_hx):
    out, _ = run(dict(x=x, W1=W1, b1=b1, W2=W2, b2=b2, init_hx=init_hx))
    return out


# revision 16
# speedup vs baseline: 1.1612x; 1.1612x over previous
"""ATR (twin-gate RNN) Trainium2 kernel.

  p = x @ W1.T + b1                       (batched GEMM over all T)
  h_t = sig(p_t+q_t)*p_t + sig(p_t-q_t)*q_t,  q_t = h_{t-1} @ W2.T + b2

Sharding: data-parallel over batch B=128 -> 16 per core across 8 NeuronCores,
zero cross-core communication. All tensors are kept on-device in a
[d-partition, batch-free] (transposed) layout; host prepares/unprepares.

Design notes (v2):
- b2 is folded into a PSUM pre-fill (constant copied into the accumulator
  ring every 4 steps); recurrence matmuls accumulate on top (start=False),
  so PSUM holds qhat = W2 h + b2 directly.
- p stored t-major [P, T, DT, BL] so all per-step reads are contiguous.
- qhat copied PSUM->SBUF once per half (ACT for A, DVE for B); the gate
  chain (s, d, t1, t2, h) then runs in fast 2x SBUF fp16 mode on DVE.
- Output h staged in SBUF, DMA'd every OG=4 steps (big transfers).
- Recurrence MMs ordered dl-outer, A-half-first: next step's first MMs
  depend only on hA.
- Junk matmuls pad the PE queue so the HAM clock gate stays at 2.4 GHz.
"""

import contextlib

import numpy as np

import concourse.bass as bass
import concourse.mybir as mybir
import concourse.tile as tile
from concourse import bacc
from concourse.bass import ts
from concourse.bass_utils import run_bass_kernel_spmd
from concourse.tile import add_dep_helper


def _chain(insts, reason):
    """Force issue order within an engine queue (no extra semaphores).

    add_dep_helper(x, y) makes x wait on y, so the later op is the first arg.
    """
    insts = [i for i in insts if i is not None]
    for a, b in zip(insts, insts[1:]):
        add_dep_helper(b.ins, a.ins, sync=False, reason=reason)
    return insts


B, T, D = 128, 256, 1024
NCORES = 8
BL = B // NCORES          # 16 batches per core
P = 128                   # partitions
DT = D // P               # 8 d-tiles
HDT = DT // 2             # 4 (half)
TW = 32                   # t-window per p-GEMM chunk (TW*BL = 512 cols)
NCH = T // TW             # 8 chunks
N_UP = 2                  # upfront t-windows for stage 1
OG = 4                    # out-DMA grouping (steps per DMA)
F16 = mybir.dt.float16
F32 = mybir.dt.float32

_CACHE = {}


def _emit(nc, xT, w1, w2, b1, b2bc4, h0, outT):
    tc = nc._tc
    SIG = mybir.ActivationFunctionType.Sigmoid
    IDENT = mybir.ActivationFunctionType.Identity
    with contextlib.ExitStack() as ctx:
        singles = ctx.enter_context(tc.tile_pool(name="singles", bufs=1))
        w1_sb = singles.tile([P, DT, D], F16)
        nc.sync.dma_start(out=w1_sb, in_=w1)
        w2_sb = singles.tile([P, DT, D], F16)
        nc.sync.dma_start(out=w2_sb, in_=w2)
        b1_sb = singles.tile([P, DT], F32)
        nc.sync.dma_start(out=b1_sb, in_=b1)
        b2bc_sb = singles.tile([P, OG, DT, BL], F32)
        nc.sync.dma_start(out=b2bc_sb, in_=b2bc4)
        h0_sb = singles.tile([P, DT, BL], F16)
        nc.sync.dma_start(out=h0_sb, in_=h0)
        # p stored t-major: per-step slice [P, DT, BL] is contiguous
        p_sb = singles.tile([P, T, DT, BL], F16)

        xT_r = xT.rearrange("(a p) t b -> a p t b", p=P)

        with contextlib.ExitStack() as rctx:
            xpool = rctx.enter_context(tc.tile_pool(name="xin", bufs=2))
            ppsum = rctx.enter_context(
                tc.tile_pool(name="pps", bufs=2, space="PSUM")
            )
            qpool = rctx.enter_context(
                tc.tile_pool(name="qps", bufs=3, space="PSUM")
            )
            gp = rctx.enter_context(tc.tile_pool(name="gp", bufs=4))
            hp = rctx.enter_context(tc.tile_pool(name="hp", bufs=2))

            xn_tiles = {}
            mmchain = [None]
            dvechain = [None]
            actchain = [None]

            def mm_order(i):
                if mmchain[0] is not None:
                    add_dep_helper(i.ins, mmchain[0].ins, sync=False,
                                   reason="mm-order")
                mmchain[0] = i

            def dve_order(i):
                if dvechain[0] is not None:
                    add_dep_helper(i.ins, dvechain[0].ins, sync=False,
                                   reason="dve-order")
                dvechain[0] = i

            def act_order(i):
                if actchain[0] is not None:
                    add_dep_helper(i.ins, actchain[0].ins, sync=False,
                                   reason="act-order")
                actchain[0] = i

            def load_xn(n, dt_range):
                if n not in xn_tiles:
                    xn_tiles[n] = xpool.tile(
                        [P, DT, TW, BL], F16, tag="xn", name=f"xn_{n}"
                    )
                for dt in dt_range:
                    nc.sync.dma_start(
                        out=xn_tiles[n][:, dt], in_=xT_r[dt, :, ts(n, TW), :]
                    )

            pps_tiles = {}

            def p_mms(n, e, dts):
                if (n, e) not in pps_tiles:
                    pps_tiles[(n, e)] = ppsum.tile(
                        [P, TW * BL], F32, tag="pp", name=f"pp_{n}_{e}"
                    )
                ps = pps_tiles[(n, e)]
                for dt in dts:
                    i_mm = nc.tensor.matmul(
                        ps,
                        lhsT=w1_sb[:, dt, ts(e, P)],
                        rhs=xn_tiles[n][:, dt].rearrange("p t b -> p (t b)"),
                        start=(dt == 0),
                        stop=(dt == DT - 1),
                    )
                    mm_order(i_mm)

            def p_copy(n, e):
                ps = pps_tiles[(n, e)]
                i_a = nc.scalar.activation(
                    out=p_sb[:, ts(n, TW), e, :],
                    in_=ps.rearrange("p (t b) -> p t b", t=TW),
                    func=IDENT,
                    bias=b1_sb[:, e : e + 1],
                    scale=1.0,
                )
                act_order(i_a)
                return i_a

            qq_tiles = {}

            def prefill(t):
                # fresh per-step accumulator halves, pre-filled with b2 so
                # the MMs accumulate qhat = W2 h + b2 directly
                qa = qpool.tile([P, HDT, BL], F32, tag="qqA", name=f"qqA_{t}")
                qb = qpool.tile([P, HDT, BL], F32, tag="qqB", name=f"qqB_{t}")
                qq_tiles[t] = (qa, qb)
                i1 = nc.vector.tensor_copy(out=qa, in_=b2bc_sb[:, 0, 0:HDT])
                dve_order(i1)
                i2 = nc.vector.tensor_copy(out=qb, in_=b2bc_sb[:, 0, HDT:DT])
                dve_order(i2)

            # ---- stage-1 upfront: chunks 0..N_UP-1
            for n in range(N_UP):
                load_xn(n, range(DT))
                for e in range(DT):
                    p_mms(n, e, range(DT))
                    p_copy(n, e)
            load_xn(N_UP, range(DT))
            prefill(0)

            # per-step schedule for interleaved stage-1 chunks
            step_mms = {}     # t -> (n, e, [dt, dt])
            step_copies = {}  # t -> (n, e)
            step_dma = {}     # t -> (n, [dts])
            K0 = 8
            for k in range((NCH - N_UP) * DT):
                n, e = N_UP + k // DT, k % DT
                for j in range(4):
                    step_mms[K0 + 4 * k + j] = (n, e, [2 * j, 2 * j + 1])
                step_copies[K0 + 4 * k + 3] = (n, e)
            for n in range(N_UP + 1, NCH):
                base = 32 * n - 72
                for dt in range(DT):
                    step_dma[base + dt] = (n, [dt])

            hprev = h0_sb  # [P, DT, BL] view for step-t rhs
            hstages = {}

            for t in range(T):
                g = t // OG
                slot = t % OG
                if slot == 0:
                    hstages[g] = hp.tile(
                        [P, OG, DT, BL], F16, tag="hst", name=f"hst_{g}"
                    )
                hst = hstages[g]
                qqA, qqB = qq_tiles[t]

                # ---- recurrence MMs: A-block (e0..3) then B-block (e4..7),
                # dl-outer so the first 16 MMs of each block depend only on
                # the previous step's A-half of h.
                for eoff, qt_ in ((0, qqA), (HDT, qqB)):
                    for dl in range(DT):
                        for el in range(HDT):
                            i_mm = nc.tensor.matmul(
                                qt_[:, el, :],
                                lhsT=w2_sb[:, dl, ts(eoff + el, P)],
                                rhs=hprev[:, dl, :],
                                start=False,
                                stop=(dl == DT - 1),
                                skip_group_check=True,
                            )
                            mm_order(i_mm)

                # ---- gate chain
                pA = p_sb[:, t, 0:HDT, :]
                pB = p_sb[:, t, HDT:DT, :]

                qcA = gp.tile([P, HDT, BL], F16, tag="qcA")
                i_qcA = nc.vector.tensor_copy(out=qcA, in_=qqA)
                qcB = gp.tile([P, HDT, BL], F16, tag="qcB")
                i_qcB = nc.vector.tensor_copy(out=qcB, in_=qqB)

                sdA = gp.tile([P, 2, HDT, BL], F16, tag="sdA")
                i_sA = nc.vector.tensor_add(sdA[:, 0], pA, qcA)
                i_dA = nc.vector.tensor_sub(sdA[:, 1], pA, qcA)
                igfgA = gp.tile([P, 2, HDT, BL], F16, tag="igfgA")
                i_sigA = nc.scalar.activation(
                    out=igfgA.rearrange("p s d b -> p (s d b)"),
                    in_=sdA.rearrange("p s d b -> p (s d b)"),
                    func=SIG,
                )
                sdB = gp.tile([P, 2, HDT, BL], F16, tag="sdB")
                i_sB = nc.vector.tensor_add(sdB[:, 0], pB, qcB)
                i_dB = nc.vector.tensor_sub(sdB[:, 1], pB, qcB)
                igfgB = gp.tile([P, 2, HDT, BL], F16, tag="igfgB")
                i_sigB = nc.scalar.activation(
                    out=igfgB.rearrange("p s d b -> p (s d b)"),
                    in_=sdB.rearrange("p s d b -> p (s d b)"),
                    func=SIG,
                )

                t1A = gp.tile([P, HDT, BL], F16, tag="t1A")
                i_t1A = nc.vector.tensor_mul(t1A, igfgA[:, 0], pA)
                t2A = gp.tile([P, HDT, BL], F16, tag="t2A")
                i_t2A = nc.vector.tensor_mul(t2A, igfgA[:, 1], qcA)
                i_hA = nc.vector.tensor_add(hst[:, slot, 0:HDT, :], t1A, t2A)
                t1B = gp.tile([P, HDT, BL], F16, tag="t1B")
                i_t1B = nc.vector.tensor_mul(t1B, igfgB[:, 0], pB)
                t2B = gp.tile([P, HDT, BL], F16, tag="t2B")
                i_t2B = nc.vector.tensor_mul(t2B, igfgB[:, 1], qcB)
                i_hB = nc.vector.tensor_add(hst[:, slot, HDT:DT, :], t1B, t2B)

                # allocate + prefill the next step's accumulators (pool
                # rotation handles the WAR hazards cleanly)
                if t + 1 < T:
                    prefill(t + 1)

                # interleaved stage-1 work
                if t in step_dma:
                    nd, dts = step_dma[t]
                    load_xn(nd, dts)
                if t in step_mms:
                    np_, ep_, dts = step_mms[t]
                    p_mms(np_, ep_, dts)
                cp = p_copy(*step_copies[t]) if t in step_copies else None

                # engine issue order
                for i in (i_qcA, i_sA, i_dA, i_qcB, i_sB, i_dB,
                          i_t1A, i_t2A, i_hA, i_t1B, i_t2B, i_hB):
                    dve_order(i)
                for i in (i_sigA, i_sigB):
                    act_order(i)

                # out DMA every OG steps
                if slot == OG - 1:
                    nc.sync.dma_start(out=outT[g], in_=hst)

                hprev = hst[:, slot]


def build():
    if "nc" in _CACHE:
        return _CACHE["nc"]
    nc = bacc.Bacc("TRN2", target_bir_lowering=False, debug=False,
                   num_devices=NCORES)
    xT = nc.dram_tensor("xT", [D, T, BL], F16, kind="ExternalInput").ap()
    w1 = nc.dram_tensor("w1", [P, DT, D], F16, kind="ExternalInput").ap()
    w2 = nc.dram_tensor("w2", [P, DT, D], F16, kind="ExternalInput").ap()
    b1 = nc.dram_tensor("b1", [P, DT], F32, kind="ExternalInput").ap()
    b2bc4 = nc.dram_tensor("b2bc4", [P, OG, DT, BL], F32,
                           kind="ExternalInput").ap()
    h0 = nc.dram_tensor("h0", [P, DT, BL], F16, kind="ExternalInput").ap()
    outT = nc.dram_tensor("outT", [T // OG, P, OG, DT, BL], F16,
                          kind="ExternalOutput").ap()
    with tile.TileContext(nc) as tc:
        nc._tc = tc
        _emit(nc, xT, w1, w2, b1, b2bc4, h0, outT)
    nc.compile()
    _CACHE["nc"] = nc
    return nc


def make_in_maps(x, W1, b1, W2, b2, init_hx):
    x = np.asarray(x, dtype=np.float32)
    W1 = np.asarray(W1, dtype=np.float32)
    b1 = np.asarray(b1, dtype=np.float32)
    W2 = np.asarray(W2, dtype=np.float32)
    b2 = np.asarray(b2, dtype=np.float32)
    init_hx = np.asarray(init_hx, dtype=np.float32)

    w1s = np.ascontiguousarray(
        W1.T.reshape(DT, P, D).transpose(1, 0, 2)
    ).astype(np.float16)  # [din, dtile, e] = W1[e, d]
    w2s = np.ascontiguousarray(
        W2.T.reshape(DT, P, D).transpose(1, 0, 2)
    ).astype(np.float16)
    b1s = np.ascontiguousarray(b1.reshape(DT, P).T)  # [e_in, e_tile]
    b2v = np.ascontiguousarray(b2.reshape(DT, P).T)  # [P, DT]
    b2bc4 = np.ascontiguousarray(
        np.broadcast_to(b2v[:, None, :, None], (P, OG, DT, BL))
    ).astype(np.float32)
    h0 = np.ascontiguousarray(
        np.broadcast_to(init_hx.reshape(DT, P).T[:, :, None], (P, DT, BL))
    ).astype(np.float16)

    in_maps = []
    for c in range(NCORES):
        xc = x[c * BL : (c + 1) * BL]  # [BL, T, D]
        xTc = np.ascontiguousarray(xc.transpose(2, 1, 0)).astype(np.float16)
        in_maps.append(
            {"xT": xTc, "w1": w1s, "w2": w2s, "b1": b1s, "b2bc4": b2bc4,
             "h0": h0}
        )
    return in_maps


def assemble(results):
    out = np.empty((B, T, D), dtype=np.float32)
    for c in range(NCORES):
        oT = results[c]["outT"]  # [T//OG, P, OG, DT, BL] f16
        out[c * BL : (c + 1) * BL] = (
            oT.transpose(4, 0, 2, 3, 1).reshape(BL, T, D).astype(np.float32)
        )
    return out


def run(inputs, trace=False):
    nc = build()
    in_maps = make_in_maps(**inputs)
    res = run_bass_kernel_spmd(nc, in_maps, list(range(NCORES)), trace=trace)
    return assemble(res.results), res


def kernel(x, W1, b1, W2, b2, init_hx):
    out, _ = run(dict(x=x, W1=W1, b1=b1, W2=W2, b2=b2, init_hx=init_hx))
    return out


# revision 21
# speedup vs baseline: 1.1692x; 1.0069x over previous
"""ATR (twin-gate RNN) Trainium2 kernel.

  p = x @ W1.T + b1                       (batched GEMM over all T)
  h_t = sig(p_t+q_t)*p_t + sig(p_t-q_t)*q_t,  q_t = h_{t-1} @ W2.T + b2

Sharding: data-parallel over batch B=128 -> 16 per core across 8 NeuronCores,
zero cross-core communication. All tensors are kept on-device in a
[d-partition, batch-free] (transposed) layout; host prepares/unprepares.

Design notes (v2):
- b2 is folded into a PSUM pre-fill (constant copied into the accumulator
  ring every 4 steps); recurrence matmuls accumulate on top (start=False),
  so PSUM holds qhat = W2 h + b2 directly.
- p stored t-major [P, T, DT, BL] so all per-step reads are contiguous.
- qhat copied PSUM->SBUF once per half (ACT for A, DVE for B); the gate
  chain (s, d, t1, t2, h) then runs in fast 2x SBUF fp16 mode on DVE.
- Output h staged in SBUF, DMA'd every OG=4 steps (big transfers).
- Recurrence MMs ordered dl-outer, A-half-first: next step's first MMs
  depend only on hA.
- Junk matmuls pad the PE queue so the HAM clock gate stays at 2.4 GHz.
"""

import contextlib

import numpy as np

import concourse.bass as bass
import concourse.mybir as mybir
import concourse.tile as tile
from concourse import bacc
from concourse.bass import ts
from concourse.bass_utils import run_bass_kernel_spmd
from concourse.tile import add_dep_helper


def _chain(insts, reason):
    """Force issue order within an engine queue (no extra semaphores).

    add_dep_helper(x, y) makes x wait on y, so the later op is the first arg.
    """
    insts = [i for i in insts if i is not None]
    for a, b in zip(insts, insts[1:]):
        add_dep_helper(b.ins, a.ins, sync=False, reason=reason)
    return insts


B, T, D = 128, 256, 1024
NCORES = 8
BL = B // NCORES          # 16 batches per core
P = 128                   # partitions
DT = D // P               # 8 d-tiles
HDT = DT // 2             # 4 (half)
TW = 32                   # t-window per p-GEMM chunk (TW*BL = 512 cols)
NCH = T // TW             # 8 chunks
N_UP = 2                  # upfront t-windows for stage 1
OG = 4                    # out-DMA grouping (steps per DMA)
F16 = mybir.dt.float16
F32 = mybir.dt.float32

_CACHE = {}


def _emit(nc, xT, w1, w2, b1, b2bc4, h0, outT):
    tc = nc._tc
    SIG = mybir.ActivationFunctionType.Sigmoid
    IDENT = mybir.ActivationFunctionType.Identity
    with contextlib.ExitStack() as ctx:
        singles = ctx.enter_context(tc.tile_pool(name="singles", bufs=1))
        w1_sb = singles.tile([P, DT, D], F16)
        nc.sync.dma_start(out=w1_sb, in_=w1)
        w2_sb = singles.tile([P, DT, D], F16)
        nc.sync.dma_start(out=w2_sb, in_=w2)
        b1_sb = singles.tile([P, DT], F32)
        nc.sync.dma_start(out=b1_sb, in_=b1)
        b2bc_sb = singles.tile([P, OG, DT, BL], F32)
        nc.sync.dma_start(out=b2bc_sb, in_=b2bc4)
        h0_sb = singles.tile([P, DT, BL], F16)
        nc.sync.dma_start(out=h0_sb, in_=h0)
        # p stored t-major: per-step slice [P, DT, BL] is contiguous
        p_sb = singles.tile([P, T, DT, BL], F16)

        xT_r = xT.rearrange("(a p) t b -> a p t b", p=P)

        with contextlib.ExitStack() as rctx:
            xpool = rctx.enter_context(tc.tile_pool(name="xin", bufs=2))
            ppsum = rctx.enter_context(
                tc.tile_pool(name="pps", bufs=2, space="PSUM")
            )
            qpool = rctx.enter_context(
                tc.tile_pool(name="qps", bufs=3, space="PSUM")
            )
            gp = rctx.enter_context(tc.tile_pool(name="gp", bufs=4))
            hp = rctx.enter_context(tc.tile_pool(name="hp", bufs=2))

            xn_tiles = {}
            mmchain = [None]
            dvechain = [None]
            actchain = [None]

            def mm_order(i):
                if mmchain[0] is not None:
                    add_dep_helper(i.ins, mmchain[0].ins, sync=False,
                                   reason="mm-order")
                mmchain[0] = i

            def dve_order(i):
                if dvechain[0] is not None:
                    add_dep_helper(i.ins, dvechain[0].ins, sync=False,
                                   reason="dve-order")
                dvechain[0] = i

            def act_order(i):
                if actchain[0] is not None:
                    add_dep_helper(i.ins, actchain[0].ins, sync=False,
                                   reason="act-order")
                actchain[0] = i

            def load_xn(n, dt_range):
                if n not in xn_tiles:
                    xn_tiles[n] = xpool.tile(
                        [P, DT, TW, BL], F16, tag="xn", name=f"xn_{n}"
                    )
                for dt in dt_range:
                    nc.sync.dma_start(
                        out=xn_tiles[n][:, dt], in_=xT_r[dt, :, ts(n, TW), :]
                    )

            pps_tiles = {}

            def p_mms(n, e, dts):
                if (n, e) not in pps_tiles:
                    pps_tiles[(n, e)] = ppsum.tile(
                        [P, TW * BL], F32, tag="pp", name=f"pp_{n}_{e}"
                    )
                ps = pps_tiles[(n, e)]
                for dt in dts:
                    i_mm = nc.tensor.matmul(
                        ps,
                        lhsT=w1_sb[:, dt, ts(e, P)],
                        rhs=xn_tiles[n][:, dt].rearrange("p t b -> p (t b)"),
                        start=(dt == 0),
                        stop=(dt == DT - 1),
                    )
                    mm_order(i_mm)

            def p_copy(n, e):
                ps = pps_tiles[(n, e)]
                i_a = nc.scalar.activation(
                    out=p_sb[:, ts(n, TW), e, :],
                    in_=ps.rearrange("p (t b) -> p t b", t=TW),
                    func=IDENT,
                    bias=b1_sb[:, e : e + 1],
                    scale=1.0,
                )
                act_order(i_a)
                return i_a

            qq_tiles = {}

            def prefill(t):
                # fresh per-step accumulator halves, pre-filled with b2 so
                # the MMs accumulate qhat = W2 h + b2 directly
                qa = qpool.tile([P, HDT, BL], F32, tag="qqA", name=f"qqA_{t}")
                qb = qpool.tile([P, HDT, BL], F32, tag="qqB", name=f"qqB_{t}")
                qq_tiles[t] = (qa, qb)
                i1 = nc.vector.tensor_copy(out=qa, in_=b2bc_sb[:, 0, 0:HDT])
                dve_order(i1)
                i2 = nc.vector.tensor_copy(out=qb, in_=b2bc_sb[:, 0, HDT:DT])
                dve_order(i2)

            # ---- stage-1 upfront: chunks 0..N_UP-1
            for n in range(N_UP):
                load_xn(n, range(DT))
                for e in range(DT):
                    p_mms(n, e, range(DT))
                    p_copy(n, e)
            load_xn(N_UP, range(DT))
            prefill(0)

            # per-step schedule for interleaved stage-1 chunks
            step_mms = {}     # t -> (n, e, [dt, dt])
            step_copies = {}  # t -> (n, e)
            step_dma = {}     # t -> (n, [dts])
            K0 = 8
            for k in range((NCH - N_UP) * DT):
                n, e = N_UP + k // DT, k % DT
                for j in range(4):
                    step_mms[K0 + 4 * k + j] = (n, e, [2 * j, 2 * j + 1])
                step_copies[K0 + 4 * k + 3] = (n, e)
            for n in range(N_UP + 1, NCH):
                base = 32 * n - 72
                for dt in range(DT):
                    step_dma[base + dt] = (n, [dt])

            hprev = h0_sb  # [P, DT, BL] view for step-t rhs
            hstages = {}

            for t in range(T):
                g = t // OG
                slot = t % OG
                if slot == 0:
                    hstages[g] = hp.tile(
                        [P, OG, DT, BL], F16, tag="hst", name=f"hst_{g}"
                    )
                hst = hstages[g]
                qqA, qqB = qq_tiles[t]

                # ---- recurrence MMs: A-block (e0..3) then B-block (e4..7),
                # dl-outer so the first 16 MMs of each block depend only on
                # the previous step's A-half of h.
                for eoff, qt_ in ((0, qqA), (HDT, qqB)):
                    for dl in range(DT):
                        for el in range(HDT):
                            i_mm = nc.tensor.matmul(
                                qt_[:, el, :],
                                lhsT=w2_sb[:, dl, ts(eoff + el, P)],
                                rhs=hprev[:, dl, :],
                                start=False,
                                stop=(dl == DT - 1),
                                skip_group_check=True,
                            )
                            mm_order(i_mm)

                # ---- gate chain
                pA = p_sb[:, t, 0:HDT, :]
                pB = p_sb[:, t, HDT:DT, :]

                sdA = gp.tile([P, 2, HDT, BL], F16, tag="sdA")
                i_sA = nc.vector.tensor_add(sdA[:, 0], pA, qqA)
                i_dA = nc.vector.tensor_sub(sdA[:, 1], pA, qqA)
                igfgA = gp.tile([P, 2, HDT, BL], F16, tag="igfgA")
                i_sigA = nc.scalar.activation(
                    out=igfgA.rearrange("p s d b -> p (s d b)"),
                    in_=sdA.rearrange("p s d b -> p (s d b)"),
                    func=SIG,
                )
                sdB = gp.tile([P, 2, HDT, BL], F16, tag="sdB")
                i_sB = nc.vector.tensor_add(sdB[:, 0], pB, qqB)
                i_dB = nc.vector.tensor_sub(sdB[:, 1], pB, qqB)
                igfgB = gp.tile([P, 2, HDT, BL], F16, tag="igfgB")
                i_sigB = nc.scalar.activation(
                    out=igfgB.rearrange("p s d b -> p (s d b)"),
                    in_=sdB.rearrange("p s d b -> p (s d b)"),
                    func=SIG,
                )

                t1A = gp.tile([P, HDT, BL], F16, tag="t1A")
                i_t1A = nc.vector.tensor_mul(t1A, igfgA[:, 0], pA)
                t2A = gp.tile([P, HDT, BL], F16, tag="t2A")
                i_t2A = nc.vector.tensor_mul(t2A, igfgA[:, 1], qqA)
                i_hA = nc.vector.tensor_add(hst[:, slot, 0:HDT, :], t1A, t2A)
                t1B = gp.tile([P, HDT, BL], F16, tag="t1B")
                i_t1B = nc.vector.tensor_mul(t1B, igfgB[:, 0], pB)
                t2B = gp.tile([P, HDT, BL], F16, tag="t2B")
                i_t2B = nc.vector.tensor_mul(t2B, igfgB[:, 1], qqB)
                i_hB = nc.vector.tensor_add(hst[:, slot, HDT:DT, :], t1B, t2B)

                # allocate + prefill the next step's accumulators (pool
                # rotation handles the WAR hazards cleanly)
                if t + 1 < T:
                    prefill(t + 1)

                # interleaved stage-1 work
                if t in step_dma:
                    nd, dts = step_dma[t]
                    load_xn(nd, dts)
                if t in step_mms:
                    np_, ep_, dts = step_mms[t]
                    p_mms(np_, ep_, dts)
                cp = p_copy(*step_copies[t]) if t in step_copies else None

                # engine issue order
                for i in (i_sA, i_dA, i_sB, i_dB,
                          i_t1A, i_t2A, i_hA, i_t1B, i_t2B, i_hB):
                    dve_order(i)
                for i in (i_sigA, i_sigB):
                    act_order(i)

                # out DMA every OG steps
                if slot == OG - 1:
                    nc.sync.dma_start(out=outT[g], in_=hst)

                hprev = hst[:, slot]


def build():
    if "nc" in _CACHE:
        return _CACHE["nc"]
    nc = bacc.Bacc("TRN2", target_bir_lowering=False, debug=False,
                   num_devices=NCORES)
    xT = nc.dram_tensor("xT", [D, T, BL], F16, kind="ExternalInput").ap()
    w1 = nc.dram_tensor("w1", [P, DT, D], F16, kind="ExternalInput").ap()
    w2 = nc.dram_tensor("w2", [P, DT, D], F16, kind="ExternalInput").ap()
    b1 = nc.dram_tensor("b1", [P, DT], F32, kind="ExternalInput").ap()
    b2bc4 = nc.dram_tensor("b2bc4", [P, OG, DT, BL], F32,
                           kind="ExternalInput").ap()
    h0 = nc.dram_tensor("h0", [P, DT, BL], F16, kind="ExternalInput").ap()
    outT = nc.dram_tensor("outT", [T // OG, P, OG, DT, BL], F16,
                          kind="ExternalOutput").ap()
    with tile.TileContext(nc) as tc:
        nc._tc = tc
        _emit(nc, xT, w1, w2, b1, b2bc4, h0, outT)
    nc.compile()
    _CACHE["nc"] = nc
    return nc


def make_in_maps(x, W1, b1, W2, b2, init_hx):
    x = np.asarray(x, dtype=np.float32)
    W1 = np.asarray(W1, dtype=np.float32)
    b1 = np.asarray(b1, dtype=np.float32)
    W2 = np.asarray(W2, dtype=np.float32)
    b2 = np.asarray(b2, dtype=np.float32)
    init_hx = np.asarray(init_hx, dtype=np.float32)

    w1s = np.ascontiguousarray(
        W1.T.reshape(DT, P, D).transpose(1, 0, 2)
    ).astype(np.float16)  # [din, dtile, e] = W1[e, d]
    w2s = np.ascontiguousarray(
        W2.T.reshape(DT, P, D).transpose(1, 0, 2)
    ).astype(np.float16)
    b1s = np.ascontiguousarray(b1.reshape(DT, P).T)  # [e_in, e_tile]
    b2v = np.ascontiguousarray(b2.reshape(DT, P).T)  # [P, DT]
    b2bc4 = np.ascontiguousarray(
        np.broadcast_to(b2v[:, None, :, None], (P, OG, DT, BL))
    ).astype(np.float32)
    h0 = np.ascontiguousarray(
        np.broadcast_to(init_hx.reshape(DT, P).T[:, :, None], (P, DT, BL))
    ).astype(np.float16)

    in_maps = []
    for c in range(NCORES):
        xc = x[c * BL : (c + 1) * BL]  # [BL, T, D]
        xTc = np.ascontiguousarray(xc.transpose(2, 1, 0)).astype(np.float16)
        in_maps.append(
            {"xT": xTc, "w1": w1s, "w2": w2s, "b1": b1s, "b2bc4": b2bc4,
             "h0": h0}
        )
    return in_maps


def assemble(results):
    out = np.empty((B, T, D), dtype=np.float32)
    for c in range(NCORES):
        oT = results[c]["outT"]  # [T//OG, P, OG, DT, BL] f16
        out[c * BL : (c + 1) * BL] = (
            oT.transpose(4, 0, 2, 3, 1).reshape(BL, T, D).astype(np.float32)
        )
    return out


def run(inputs, trace=False):
    nc = build()
    in_maps = make_in_maps(**inputs)
    res = run_bass_kernel_spmd(nc, in_maps, list(range(NCORES)), trace=trace)
    return assemble(res.results), res


def kernel(x, W1, b1, W2, b2, init_hx):
    out, _ = run(dict(x=x, W1=W1, b1=b1, W2=W2, b2=b2, init_hx=init_hx))
    return out
